# revision 1
# baseline (speedup 1.0000x reference)
"""GCN block (GraphConv + BatchNorm1d + ReLU) on 8 Trainium2 NeuronCores.

Strategy (per sharding hint): partition nodes (and incident edges) across the
8 cores; replicate W/b/gamma/beta; all-reduce BN batch statistics.

Per core k (owns dst nodes [k*NPC, (k+1)*NPC)):
  1. h_k = (x_k @ W) * rsqrt(clip(deg_out_k,1))           (PE matmul, fp32)
  2. AllGather h (bf16) -> full h table in every core's HBM
  3. For each 128-node dst group, gather h[src] rows of the group's edges
     (dma_gather, bf16, batched) and segment-sum them with one-hot matmuls
     M^T @ G accumulated in PSUM (avoids dma_scatter_add, which loses
     updates on duplicate indices - verified on HW).
  4. relu(agg * rsqrt(clip(deg_in,1)) + b); local BN sums; AllReduce sums;
     y = (h - mu) * rsqrt(var+eps) * gamma + beta.

Host-side work is limited to integer index bookkeeping (bucketing edges by
(core, src-bank, dst-group), degree counting) and layout transforms (x^T,
int16 gather indices). All floating-point math runs on device.

Edges are bucketed by src bank (4 banks of N/4 rows) because dma_gather
indices are int16 (< 32768). Bucket sizes are padded to a structure shared
by all 8 cores so a single SPMD NEFF serves every core; pad slots gather row
0 of the bank and carry a dst offset of 255 -> their one-hot column is all
zeros, so they contribute exactly 0.
"""
import math
import os
import sys

sys.path.insert(0, "/opt/trn_rl_repo")

import numpy as np

import concourse.bacc as bacc
import concourse.bass as bass
import concourse.mybir as mybir
import concourse.tile as tile
from concourse import bass_utils

F32 = mybir.dt.float32
BF16 = mybir.dt.bfloat16
I16 = mybir.dt.int16

CFG = dict(
    N=100000,
    E=1600000,
    IN=256,
    OUT=128,
    NCORES=8,
    GRP=128,          # dst nodes per segment group (= psum partition dim)
    NBANKS=4,         # src banks (bank rows must stay < 32768 for int16 idx)
    BATCH_BLOCKS=40,  # gather batch size in 128-edge blocks
    EPS=1e-5,
    TRACE=False,
)

LAST_RESULTS = None  # set by kernel() for test harness introspection
LAST_NC = None
LAST_RUN_S = None


def _ceil_div(a, b):
    return (a + b - 1) // b


def _wrap16(idx, ncols):
    """int16 idx list -> [128, ncols] tile: idx i at [i%16, i//16], replicated
    8x across the 16-partition groups (one copy per GpSimd Q7 core)."""
    n = idx.shape[0]
    assert n == ncols * 16
    w = np.ascontiguousarray(idx.reshape(ncols, 16).T)
    return np.tile(w, (8, 1))


def _preprocess(cfg, src, dst):
    """Bucket edges by (owner core, src bank, dst group); build per-core
    gather-index / dst-offset arrays and the shared block structure."""
    N, E = cfg["N"], cfg["E"]
    C, NBANKS, GRP = cfg["NCORES"], cfg["NBANKS"], cfg["GRP"]
    NPC = N // C
    NG = _ceil_div(NPC, GRP)
    assert NPC % NBANKS == 0
    QROWS = NPC // NBANKS          # rows per quarter of a core's shard
    BANKROWS = QROWS * C           # rows per bank table (one AllGather output)
    assert BANKROWS < 32768

    src = src.astype(np.int64)
    dst = dst.astype(np.int64)
    deg_out = np.bincount(src, minlength=N).astype(np.float32)
    deg_in = np.bincount(dst, minlength=N).astype(np.float32)

    owner = dst // NPC
    bank = (src % NPC) // QROWS    # quarter index within the source's shard
    grp = (dst % NPC) // GRP
    key = (owner * NBANKS + bank) * NG + grp
    order = np.argsort(key, kind="stable")
    s_src = src[order]
    s_dst = dst[order]
    s_key = key[order]

    counts = np.bincount(key, minlength=C * NBANKS * NG).reshape(C, NBANKS, NG)
    P = counts.max(axis=0)  # [NBANKS, NG]
    P = ((P + 127) // 128) * 128
    P[0] = np.maximum(P[0], 128)  # bank-0 run always exists (initializes agg)

    nidx_tot = int(P.sum())
    nb_tot = nidx_tot // 128
    # stream order: group-chunks outer, banks inner -> the ReLU/BN stage of a
    # chunk's groups can overlap later chunks' gathers
    GC = cfg.get("GCHUNK", 13)
    chunks = [list(range(c, min(c + GC, NG))) for c in range(0, NG, GC)]
    run_seq = [(b, g) for ch in chunks for b in range(NBANKS) for g in ch]
    run_off = np.zeros((NBANKS, NG), np.int64)
    pos = 0
    for b, g in run_seq:
        run_off[b, g] = pos
        pos += P[b, g]

    # boundaries of each (k, b, g) bucket in the sorted edge stream
    bkeys = (np.arange(C)[:, None, None] * NBANKS + np.arange(NBANKS)[None, :, None]) * NG + np.arange(NG)[None, None, :]
    starts = np.searchsorted(s_key, bkeys.ravel()).reshape(C, NBANKS, NG)
    ends = np.searchsorted(s_key, bkeys.ravel(), side="right").reshape(C, NBANKS, NG)

    gidx_cores = []
    dstoff_cores = []
    for k in range(C):
        gidx = np.zeros(nidx_tot, np.int16)
        doff = np.full(nidx_tot, 255.0, np.float32)
        for b in range(NBANKS):
            for g in range(NG):
                s, e = starts[k, b, g], ends[k, b, g]
                cnt = e - s
                if cnt == 0:
                    continue
                p0 = run_off[b, g]
                gidx[p0 : p0 + cnt] = (
                    (s_src[s:e] // NPC) * QROWS + (s_src[s:e] % NPC) % QROWS
                ).astype(np.int16)
                doff[p0 : p0 + cnt] = ((s_dst[s:e] % NPC) - g * GRP).astype(np.float32)
        gidx_cores.append(_wrap16(gidx, nidx_tot // 16))
        # dstoff tile [128, nb_tot]: col t = offsets of block t's 128 edges
        dstoff_cores.append(np.ascontiguousarray(doff.reshape(nb_tot, 128).T))

    # shared static block structure: per block t -> (bank, group, start, stop)
    blocks = []
    for b, g in run_seq:
        nb = P[b, g] // 128
        for j in range(nb):
            blocks.append((b, g, j == 0, j == nb - 1))

    # gather batches: consecutive blocks within one bank, <= BATCH_BLOCKS
    batches = []  # (bank, first_block, n_blocks)
    t = 0
    while t < len(blocks):
        b = blocks[t][0]
        n = 1
        while (
            t + n < len(blocks)
            and blocks[t + n][0] == b
            and n < cfg["BATCH_BLOCKS"]
        ):
            n += 1
        batches.append((b, t, n))
        t += n

    meta = dict(
        NPC=NPC,
        NG=NG,
        BANKROWS=BANKROWS,
        QROWS=QROWS,
        nidx_tot=nidx_tot,
        nb_tot=nb_tot,
        blocks=blocks,
        batches=batches,
        deg_out=deg_out,
        deg_in=deg_in,
    )
    return meta, gidx_cores, dstoff_cores


def _tile_major(vec, NG, GRP, pad_val):
    """[NPC] -> [GRP, NG]: entry (p, m) = vec[m*GRP + p], padded."""
    out = np.full((NG * GRP,), pad_val, vec.dtype)
    out[: vec.shape[0]] = vec
    return np.ascontiguousarray(out.reshape(NG, GRP).T)


def _build_nc(cfg, meta):
    N, IN, OUT, C = cfg["N"], cfg["IN"], cfg["OUT"], cfg["NCORES"]
    GRP, NBANKS = cfg["GRP"], cfg["NBANKS"]
    NPC, NG, BANKROWS = meta["NPC"], meta["NG"], meta["BANKROWS"]
    QROWS = meta["QROWS"]
    nidx_tot, nb_tot = meta["nidx_tot"], meta["nb_tot"]
    blocks, batches = meta["blocks"], meta["batches"]
    XK = _ceil_div(IN, 128)
    assert OUT == 128 and GRP == 128
    last_w = NPC - (NG - 1) * GRP  # valid rows in the last group

    nc = bacc.Bacc(
        "TRN2", target_bir_lowering=False, debug=False, num_devices=C
    )

    # ---- external inputs ----
    xt = [
        nc.dram_tensor(f"xt{j}", [128, NPC], BF16, kind="ExternalInput")
        for j in range(XK)
    ]
    wt = [
        nc.dram_tensor(f"wt{j}", [128, OUT], BF16, kind="ExternalInput")
        for j in range(XK)
    ]
    gidx_d = nc.dram_tensor("gidx", [128, nidx_tot // 16], I16, kind="ExternalInput")
    doff_d = nc.dram_tensor("doff", [128, nb_tot], F32, kind="ExternalInput")
    dego_d = nc.dram_tensor("dego", [128, NG], F32, kind="ExternalInput")
    degi_d = nc.dram_tensor("degi", [128, NG], F32, kind="ExternalInput")
    bt_d = nc.dram_tensor("bt", [128, OUT], F32, kind="ExternalInput")
    iota_d = nc.dram_tensor("iota", [128, GRP], BF16, kind="ExternalInput")
    gm_d = nc.dram_tensor("gm", [1, OUT], F32, kind="ExternalInput")
    bb_d = nc.dram_tensor("bb", [1, OUT], F32, kind="ExternalInput")
    onesc_d = nc.dram_tensor("onesc", [128, 1], F32, kind="ExternalInput")
    onest_d = nc.dram_tensor("onest", [128, 1], F32, kind="ExternalInput")
    onesr_d = nc.dram_tensor("onesr", [1, 128], F32, kind="ExternalInput")

    ypad_d = nc.dram_tensor("ypad", [NG * GRP, OUT], F32, kind="ExternalOutput")

    with tile.TileContext(nc) as tc:
        with (
            tc.tile_pool(name="const", bufs=1) as cpool,
            tc.tile_pool(name="dram", bufs=1, space="DRAM") as dpool,
            tc.tile_pool(name="agg", bufs=1) as apool,
            tc.tile_pool(name="gath", bufs=3) as gpool,
            tc.tile_pool(name="mpool", bufs=6) as mpool,
            tc.tile_pool(name="etmp", bufs=4) as epool,
            tc.tile_pool(name="gtmp", bufs=4) as gpool2,
            tc.tile_pool(name="psum", bufs=3, space="PSUM") as ppool,
            tc.tile_pool(name="pstat", bufs=1, space="PSUM") as pspool,
        ):
            # ---- constants / small tiles ----
            iota_t = cpool.tile([128, GRP], BF16)
            bt_t = cpool.tile([128, OUT], F32)
            dego_t = cpool.tile([128, NG], F32)
            degi_t = cpool.tile([128, NG], F32)
            nsrc_t = cpool.tile([128, NG], F32)
            ndst_t = cpool.tile([128, NG], F32)
            gm_t = cpool.tile([1, OUT], F32)
            bb_t = cpool.tile([1, OUT], F32)
            onesc_t = cpool.tile([128, 1], F32)
            onest_t = cpool.tile([128, 1], F32)
            onesr_t = cpool.tile([1, 128], F32)
            gidx_t = cpool.tile([128, nidx_tot // 16], I16)
            doff_t = cpool.tile([128, nb_tot], F32)

            nc.sync.dma_start(iota_t[:], iota_d[:])
            nc.sync.dma_start(bt_t[:], bt_d[:])
            nc.sync.dma_start(dego_t[:], dego_d[:])
            nc.sync.dma_start(degi_t[:], degi_d[:])
            nc.sync.dma_start(gm_t[:], gm_d[:])
            nc.sync.dma_start(bb_t[:], bb_d[:])
            nc.sync.dma_start(onesc_t[:], onesc_d[:])
            nc.sync.dma_start(onest_t[:], onest_d[:])
            nc.sync.dma_start(onesr_t[:], onesr_d[:])
            nc.sync.dma_start(gidx_t[:], gidx_d[:])
            nc.sync.dma_start(doff_t[:], doff_d[:])

            # norms: rsqrt(max(deg, 1))
            for deg_t, norm_t in ((dego_t, nsrc_t), (degi_t, ndst_t)):
                nc.vector.tensor_scalar(
                    norm_t[:], deg_t[:], 1.0, None, op0=mybir.AluOpType.max
                )
                nc.vector.reciprocal(norm_t[:], norm_t[:])
                nc.scalar.activation(
                    norm_t[:], norm_t[:], mybir.ActivationFunctionType.Sqrt
                )

            # internal DRAM for collectives (quartered for B/C/D pipelining)
            h_my_qs = [
                dpool.tile([QROWS, OUT], BF16, name=f"h_my_{q}")
                for q in range(NBANKS)
            ]
            _aspace = "Local" if cfg.get("NOCC") else "Shared"
            h_all_qs = [
                dpool.tile([BANKROWS, OUT], BF16, addr_space=_aspace, name=f"h_all_{q}")
                for q in range(NBANKS)
            ]
            stats_in = dpool.tile([1, 2 * OUT], F32)
            stats_out = dpool.tile([1, 2 * OUT], F32, addr_space=_aspace)

            agg_t = apool.tile([128, NG, OUT], F32)

            # ---- stage B: h = (x @ W) * norm_src, cast bf16, store to HBM
            with tc.tile_pool(name="xw", bufs=1) as xwp, tc.tile_pool(
                name="hbf", bufs=4
            ) as hbp:
                xts = []
                wts = []
                for j in range(XK):
                    xtile = xwp.tile([128, NPC], BF16, name=f"xt_s{j}")
                    wtile = xwp.tile([128, OUT], BF16, name=f"wt_s{j}")
                    nc.sync.dma_start(xtile[:], xt[j][:])
                    nc.sync.dma_start(wtile[:], wt[j][:])
                    xts.append(xtile)
                    wts.append(wtile)
                for m in range(NG):
                    w = GRP if m < NG - 1 else last_w
                    ps = ppool.tile([128, OUT], F32, tag="hps")
                    for j in range(XK):
                        nc.tensor.matmul(
                            ps[:w, :],
                            xts[j][:, m * GRP : m * GRP + w],
                            wts[j][:, :],
                            start=(j == 0),
                            stop=(j == XK - 1),
                        )
                    hb = hbp.tile([128, OUT], BF16, tag="hb")
                    nc.scalar.activation(
                        hb[:w, :],
                        ps[:w, :],
                        mybir.ActivationFunctionType.Copy,
                        scale=nsrc_t[:w, m : m + 1],
                    )
                    r0 = m * GRP
                    r1 = r0 + w
                    q0 = r0 // QROWS
                    q1 = (r1 - 1) // QROWS
                    for q in range(q0, q1 + 1):
                        a = max(r0, q * QROWS)
                        z = min(r1, (q + 1) * QROWS)
                        nc.sync.dma_start(
                            h_my_qs[q][a - q * QROWS : z - q * QROWS, :],
                            hb[a - r0 : z - r0, :],
                        )

            # ---- stage C: quartered AllGather (pipelines with B and D) ----
            for q in range(NBANKS):
                if cfg.get("NOCC"):
                    rep = (
                        h_my_qs[q][:]
                        .rearrange("(o r) f -> o r f", o=1)
                        .to_broadcast((C, QROWS, OUT))
                    )
                    nc.sync.dma_start(
                        h_all_qs[q][:].rearrange("(o r) f -> o r f", o=C), rep
                    )
                else:
                    nc.gpsimd.collective_compute(
                        "AllGather",
                        mybir.AluOpType.bypass,
                        replica_groups=[list(range(C))],
                        ins=[h_my_qs[q][:]],
                        outs=[h_all_qs[q][:]],
                    )

            # ---- stage D: gather + one-hot matmul segmented sum ----
            stages = cfg.get("STAGES", "BCDEFG")
            if "D" not in stages or cfg.get("DSUB", 3) < 3:
                nc.gpsimd.memset(agg_t[:], 0.0)
            if "D" in stages:
              if True:
                  ps_run = None
                  bmax = max(nb for _, _, nb in batches)
                  for bank, t0, nblk in batches:
                      Gt = gpool.tile([128, bmax, OUT], BF16, tag="G")
                      nc.gpsimd.dma_gather(
                          Gt[:, :nblk, :],
                          h_all_qs[bank][:],
                          gidx_t[:, t0 * 8 : (t0 + nblk) * 8],
                          nblk * 128,
                          nblk * 128,
                          OUT,
                          single_packet=False,
                      )
                      for j in range(nblk):
                          if cfg.get("DSUB", 3) < 2:
                              continue
                          t = t0 + j
                          b, g, is_start, is_stop = blocks[t]
                          Mt = mpool.tile([128, GRP], BF16, tag="M")
                          nc.vector.tensor_scalar(
                              Mt[:],
                              iota_t[:],
                              doff_t[:, t : t + 1],
                              None,
                              op0=mybir.AluOpType.is_equal,
                          )
                          if cfg.get("DSUB", 3) < 3:
                              continue
                          if is_start:
                              ps_run = ppool.tile([128, OUT], F32, tag="aggps")
                          nc.tensor.matmul(
                              ps_run[:],
                              Mt[:],
                              Gt[:, j, :],
                              start=is_start,
                              stop=is_stop,
                          )
                          if is_stop:
                              if b == 0:
                                  nc.scalar.activation(
                                      agg_t[:, g, :],
                                      ps_run[:],
                                      mybir.ActivationFunctionType.Copy,
                                  )
                              else:
                                  nc.vector.tensor_tensor(
                                      agg_t[:, g, :],
                                      agg_t[:, g, :],
                                      ps_run[:],
                                      op=mybir.AluOpType.add,
                                  )

            # ---- stage E: relu(agg*norm_dst + b); BN partial sums ----
            ps_sum = pspool.tile([1, OUT], F32, name="ps_sum")
            ps_sq = pspool.tile([1, OUT], F32, name="ps_sq")
            if "E" in stages:
              if True:
                  for g in range(NG):
                      tmp = epool.tile([128, OUT], F32, tag="etmp")
                      nc.vector.scalar_tensor_tensor(
                          tmp[:],
                          agg_t[:, g, :],
                          ndst_t[:, g : g + 1],
                          bt_t[:],
                          op0=mybir.AluOpType.mult,
                          op1=mybir.AluOpType.add,
                      )
                      nc.scalar.activation(
                          agg_t[:, g, :], tmp[:], mybir.ActivationFunctionType.Relu
                      )
                      ones = onesc_t if g < NG - 1 else onest_t
                      nc.tensor.matmul(
                          ps_sum[:],
                          ones[:],
                          agg_t[:, g, :],
                          start=(g == 0),
                          stop=(g == NG - 1),
                      )
                      sq = epool.tile([128, OUT], F32, tag="esq")
                      nc.scalar.activation(
                          sq[:], agg_t[:, g, :], mybir.ActivationFunctionType.Square
                      )
                      nc.tensor.matmul(
                          ps_sq[:],
                          ones[:],
                          sq[:],
                          start=(g == 0),
                          stop=(g == NG - 1),
                      )

            # ---- stage F: AllReduce BN stats; build affine S/T tiles ----
            S_t = cpool.tile([128, OUT], F32)
            T_t = cpool.tile([128, OUT], F32)
            if "F" not in stages:
                nc.gpsimd.memset(S_t[:], 1.0)
                nc.gpsimd.memset(T_t[:], 0.0)
            if "F" in stages:
              st_sb = cpool.tile([1, 2 * OUT], F32)
              nc.scalar.activation(
                  st_sb[:, 0:OUT], ps_sum[:], mybir.ActivationFunctionType.Copy
              )
              nc.scalar.activation(
                  st_sb[:, OUT : 2 * OUT], ps_sq[:], mybir.ActivationFunctionType.Copy
              )
              nc.sync.dma_start(stats_in[:], st_sb[:])
              if cfg.get("NOCC"):
                  nc.sync.dma_start(stats_out[:], stats_in[:])
              else:
                  nc.gpsimd.collective_compute(
                      "AllReduce",
                      mybir.AluOpType.add,
                      replica_groups=[list(range(C))],
                      ins=[stats_in[:]],
                      outs=[stats_out[:]],
                  )
              st_rb = cpool.tile([1, 2 * OUT], F32)
              nc.sync.dma_start(st_rb[:], stats_out[:])

              mu = cpool.tile([1, OUT], F32)
              ex2 = cpool.tile([1, OUT], F32)
              var = cpool.tile([1, OUT], F32)
              srow = cpool.tile([1, OUT], F32)
              trow = cpool.tile([1, OUT], F32)
              inv_n = 1.0 / float(N)
              nc.scalar.activation(
                  mu[:], st_rb[:, 0:OUT], mybir.ActivationFunctionType.Copy, scale=inv_n
              )
              nc.scalar.activation(
                  ex2[:], st_rb[:, OUT : 2 * OUT], mybir.ActivationFunctionType.Copy, scale=inv_n
              )
              nc.scalar.activation(
                  var[:], mu[:], mybir.ActivationFunctionType.Square
              )
              nc.vector.tensor_sub(var[:], ex2[:], var[:])
              # var <- rsqrt(var + eps) (ACT Rsqrt is banned for accuracy)
              nc.scalar.activation(
                  var[:],
                  var[:],
                  mybir.ActivationFunctionType.Copy,
                  bias=float(cfg["EPS"]),
              )
              nc.vector.reciprocal(var[:], var[:])
              nc.scalar.activation(
                  var[:], var[:], mybir.ActivationFunctionType.Sqrt
              )
              nc.vector.tensor_mul(srow[:], gm_t[:], var[:])
              nc.vector.tensor_mul(trow[:], mu[:], srow[:])
              nc.vector.tensor_sub(trow[:], bb_t[:], trow[:])

              ps_S = ppool.tile([128, OUT], F32, tag="aggps", name="ps_S")
              ps_T = ppool.tile([128, OUT], F32, tag="aggps", name="ps_T")
              nc.tensor.matmul(ps_S[:], onesr_t[:], srow[:], start=True, stop=True)
              nc.tensor.matmul(ps_T[:], onesr_t[:], trow[:], start=True, stop=True)
              nc.scalar.activation(
                  S_t[:], ps_S[:], mybir.ActivationFunctionType.Copy
              )
              nc.scalar.activation(
                  T_t[:], ps_T[:], mybir.ActivationFunctionType.Copy
              )

            # ---- stage G: y = hrelu * S + T, write out ----
            if True:
                for g in range(NG):
                    tmp = gpool2.tile([128, OUT], F32, tag="gtmp")
                    nc.vector.tensor_mul(tmp[:], agg_t[:, g, :], S_t[:])
                    nc.vector.tensor_add(agg_t[:, g, :], tmp[:], T_t[:])
                ypad_view = ypad_d[:].rearrange("(g p) f -> p g f", p=128)
                nc.sync.dma_start(ypad_view, agg_t[:, :, :])

    nc.compile()
    return nc


def kernel(x, src, dst, W, b, gamma, beta):
    global LAST_RESULTS
    cfg = CFG
    N, E, IN, OUT, C = cfg["N"], cfg["E"], cfg["IN"], cfg["OUT"], cfg["NCORES"]
    GRP = cfg["GRP"]
    assert x.shape == (N, IN) and W.shape == (IN, OUT)
    assert src.shape == (E,) and dst.shape == (E,)

    meta, gidx_cores, dstoff_cores = _preprocess(cfg, src, dst)
    NPC, NG = meta["NPC"], meta["NG"]
    XK = _ceil_div(IN, 128)
    last_w = NPC - (NG - 1) * GRP

    nc = _build_nc(cfg, meta)

    xT = np.ascontiguousarray(np.asarray(x, np.float32).T)  # [IN, N]
    Wn = np.asarray(W, np.float32)
    import ml_dtypes

    iota = np.tile(
        np.arange(GRP, dtype=np.float32)[None, :], (128, 1)
    ).astype(ml_dtypes.bfloat16)
    bt = np.tile(np.asarray(b, np.float32)[None, :], (128, 1))
    onesc = np.ones((128, 1), np.float32)
    onest = np.zeros((128, 1), np.float32)
    onest[:last_w] = 1.0
    onesr = np.ones((1, 128), np.float32)
    gm = np.asarray(gamma, np.float32)[None, :]
    bb = np.asarray(beta, np.float32)[None, :]

    in_maps = []
    for k in range(C):
        im = {
            "gidx": gidx_cores[k],
            "doff": dstoff_cores[k],
            "dego": _tile_major(
                meta["deg_out"][k * NPC : (k + 1) * NPC], NG, GRP, np.float32(1.0)
            ),
            "degi": _tile_major(
                meta["deg_in"][k * NPC : (k + 1) * NPC], NG, GRP, np.float32(1.0)
            ),
            "bt": bt,
            "iota": iota,
            "gm": gm,
            "bb": bb,
            "onesc": onesc,
            "onest": onest,
            "onesr": onesr,
        }
        for j in range(XK):
            im[f"xt{j}"] = np.ascontiguousarray(
                xT[j * 128 : (j + 1) * 128, k * NPC : (k + 1) * NPC]
            ).astype(ml_dtypes.bfloat16)
            im[f"wt{j}"] = np.ascontiguousarray(
                Wn[j * 128 : (j + 1) * 128, :]
            ).astype(ml_dtypes.bfloat16)
        in_maps.append(im)

    if cfg.get("SIM"):
        from concourse.bass_interp import MultiCoreSim

        sim = MultiCoreSim(nc, num_cores=C)
        for k, core_sim in sim.cores.items():
            for name, val in in_maps[k].items():
                core_sim.tensor(name)[:] = val
        sim.simulate()
        y = np.empty((N, OUT), np.float32)
        for k in range(C):
            y[k * NPC : (k + 1) * NPC] = sim.cores[k].tensor("ypad")[:NPC]
        return y

    global LAST_NC, LAST_RUN_S
    LAST_NC = nc
    import time as _time

    _t0 = _time.time()
    res = bass_utils.run_bass_kernel_spmd(
        nc,
        in_maps,
        core_ids=list(range(C)),
        trace=cfg.get("TRACE", False),
    )
    LAST_RUN_S = _time.time() - _t0
    LAST_RESULTS = res

    y = np.empty((N, OUT), np.float32)
    for k in range(C):
        y[k * NPC : (k + 1) * NPC] = res.results[k]["ypad"][:NPC]
    return y



# revision 26
# speedup vs baseline: 1.5033x; 1.5033x over previous
"""GCN block (GraphConv + BatchNorm1d + ReLU) on 8 Trainium2 NeuronCores.

Strategy (per sharding hint): partition nodes (and incident edges) across the
8 cores; replicate W/b/gamma/beta; all-reduce BN batch statistics.

Per core k (owns dst nodes [k*NPC, (k+1)*NPC)):
  1. h_k = (x_k @ W) * rsqrt(clip(deg_out_k,1))           (PE matmul, fp32)
  2. ONE AllGather of h (bf16) -> full h table [N, OUT] in every core's HBM.
     The 4 int16-indexable "bank" tables are *interleaved strided views* of
     the AG output: bank b = nodes with (n % NPC) % 4 == b, gather row
     j = owner*(NPC/4) + (n % NPC)//4, HBM row = 4*j + b (elem_step=4 rows).
     Interleaving makes the bank-row -> table-row map linear, so one big
     collective (cheap) serves four int16-indexed gather tables.
  3. For each (bank, dst-group) run of edges, gather h[src] rows (dma_gather,
     bf16, one batched gather per (bank, chunk-of-groups)) and segment-sum
     them with one-hot matmuls M^T @ G accumulated in PSUM. Each dst group
     keeps ONE psum tile accumulated across all 4 banks (start on its bank-0
     run, stop on its bank-3 run) - no SBUF merge adds.
  4. relu(psum * rsqrt(clip(deg_in,1)) [+ b]) via ACT directly from PSUM
     (bf16 out); BN sums via ones-matmuls; AllReduce sums; y = h*S + T with
     S = gamma*rsqrt(var+eps), T = beta - mu*S; y cast bf16->f32 during the
     output DMA (SWDGE).

Host-side work is limited to integer index bookkeeping (bucketing edges by
(core, src-bank, dst-group), degree counting) and layout transforms (x^T,
int16 gather indices). All floating-point math runs on device.

Bucket sizes are padded to a structure shared by all 8 cores so a single
SPMD NEFF serves every core; pad slots gather row 0 of the bank view and
carry a dst offset of 255 -> their one-hot column is all zeros -> contribute
exactly 0. Edges are sorted by gather row within each bucket for HBM
locality.
"""
import math
import os
import sys

sys.path.insert(0, "/opt/trn_rl_repo")

import numpy as np

import concourse.bacc as bacc
import concourse.bass as bass
import concourse.mybir as mybir
import concourse.tile as tile
from concourse import bass_utils

F32 = mybir.dt.float32
BF16 = mybir.dt.bfloat16
I16 = mybir.dt.int16

CFG = dict(
    N=100000,
    E=1600000,
    IN=256,
    OUT=128,
    NCORES=8,
    GRP=128,          # dst nodes per segment group (= psum partition dim)
    NBANKS=4,         # interleaved src banks (bank rows must be < 32768)
    GCHUNK=8,         # groups per chunk (gather batch granularity)
    EPS=1e-5,
    TRACE=False,
)

LAST_RESULTS = None  # set by kernel() for test harness introspection
LAST_NC = None
LAST_RUN_S = None


def _ceil_div(a, b):
    return (a + b - 1) // b


def _wrap16(idx, ncols):
    """int16 idx list -> [128, ncols] tile: idx i at [i%16, i//16], replicated
    8x across the 16-partition groups (one copy per GpSimd Q7 core)."""
    n = idx.shape[0]
    assert n == ncols * 16
    w = np.ascontiguousarray(idx.reshape(ncols, 16).T)
    return np.tile(w, (8, 1))


def _preprocess(cfg, src, dst):
    """Bucket edges by (owner core, interleaved src bank, dst group); build
    per-core gather-index / dst-offset arrays and the shared run structure."""
    N, E = cfg["N"], cfg["E"]
    C, NBANKS, GRP, GC = cfg["NCORES"], cfg["NBANKS"], cfg["GRP"], cfg["GCHUNK"]
    NPC = N // C
    NG = _ceil_div(NPC, GRP)
    NPCP = NG * GRP                # padded nodes per core (x cols zero-padded)
    assert NPCP % NBANKS == 0
    QB = NPCP // NBANKS            # gather rows per owner per bank view
    BANKROWS = QB * C              # rows per bank view of one AG-half output
    assert BANKROWS < 32768

    src = src.astype(np.int64)
    dst = dst.astype(np.int64)
    deg_out = np.bincount(src, minlength=N).astype(np.float32)
    deg_in = np.bincount(dst, minlength=N).astype(np.float32)

    owner = dst // NPC
    loc = src % NPC
    bank = loc % NBANKS            # interleaved bank of the source
    grow = (src // NPC) * QB + loc // NBANKS   # gather row within bank view
    assert grow.max() < 32768
    grp = (dst % NPC) // GRP
    key = (owner * NBANKS + bank) * NG + grp
    # sort by bucket, then by gather row inside the bucket (HBM locality)
    order = np.lexsort((grow, key))
    s_grow = grow[order]
    s_dst = dst[order]
    s_key = key[order]

    counts = np.bincount(key, minlength=C * NBANKS * NG).reshape(C, NBANKS, NG)
    P = counts.max(axis=0)  # [NBANKS, NG] shared run sizes
    P = ((P + 127) // 128) * 128
    P = np.maximum(P, 128)  # every (b,g) run structurally exists

    nidx_tot = int(P.sum())
    nb_tot = nidx_tot // 128
    # stream order: chunks outer, banks inner, groups innermost. Each group's
    # psum accumulates across its 4 bank runs within the chunk pass.
    chunks = [list(range(c, min(c + GC, NG))) for c in range(0, NG, GC)]
    # two phases: banks {0,2} (even AG half), then {1,3} (odd half)
    phases = [(0, 2), (1, 3)]
    run_seq = [
        (b, g) for ph in phases for ch in chunks for b in ph for g in ch
    ]
    run_off = np.zeros((NBANKS, NG), np.int64)
    pos = 0
    for b, g in run_seq:
        run_off[b, g] = pos
        pos += P[b, g]

    # boundaries of each (k, b, g) bucket in the sorted edge stream
    bkeys = (
        np.arange(C)[:, None, None] * NBANKS + np.arange(NBANKS)[None, :, None]
    ) * NG + np.arange(NG)[None, None, :]
    starts = np.searchsorted(s_key, bkeys.ravel()).reshape(C, NBANKS, NG)
    ends = np.searchsorted(s_key, bkeys.ravel(), side="right").reshape(C, NBANKS, NG)

    gidx_cores = []
    dstoff_cores = []
    for k in range(C):
        gidx = np.zeros(nidx_tot, np.int16)
        doff = np.full(nidx_tot, 255.0, np.float32)
        for b in range(NBANKS):
            for g in range(NG):
                s, e = starts[k, b, g], ends[k, b, g]
                cnt = e - s
                if cnt == 0:
                    continue
                p0 = run_off[b, g]
                gidx[p0 : p0 + cnt] = s_grow[s:e].astype(np.int16)
                # pad slots re-gather the bucket's last row (HBM page hit)
                gidx[p0 + cnt : p0 + P[b, g]] = gidx[p0 + cnt - 1]
                doff[p0 : p0 + cnt] = ((s_dst[s:e] % NPC) - g * GRP).astype(
                    np.float32
                )
        gidx_cores.append(_wrap16(gidx, nidx_tot // 16))
        # dstoff tile [128, nb_tot]: col t = offsets of block t's 128 edges
        dstoff_cores.append(np.ascontiguousarray(doff.reshape(nb_tot, 128).T))

    # shared static structures -------------------------------------------
    # per (b, g): first block index and block count of its run
    run_blk = {}
    for b, g in run_seq:
        run_blk[(b, g)] = (int(run_off[b, g]) // 128, int(P[b, g]) // 128)
    # gather units: one per (phase, chunk, bank) -> contiguous block range
    units = []  # (bank, first_block, n_blocks) in stream order
    t = 0
    for ph in phases:
        for ci, ch in enumerate(chunks):
            for b in ph:
                nb = int(sum(P[b, g] for g in ch)) // 128
                units.append((b, t, nb))
                t += nb
    assert t == nb_tot

    meta = dict(
        NPC=NPC,
        NPCP=NPCP,
        NG=NG,
        QB=QB,
        BANKROWS=BANKROWS,
        nidx_tot=nidx_tot,
        nb_tot=nb_tot,
        run_blk=run_blk,
        units=units,
        chunks=chunks,
        run_seq=run_seq,
        deg_out=deg_out,
        deg_in=deg_in,
    )
    return meta, gidx_cores, dstoff_cores


def _tile_major(vec, NG, GRP, pad_val):
    """[NPC] -> [GRP, NG]: entry (p, m) = vec[m*GRP + p], padded."""
    out = np.full((NG * GRP,), pad_val, vec.dtype)
    out[: vec.shape[0]] = vec
    return np.ascontiguousarray(out.reshape(NG, GRP).T)


def _build_nc(cfg, meta, b_nonzero=False):
    N, IN, OUT, C = cfg["N"], cfg["IN"], cfg["OUT"], cfg["NCORES"]
    GRP, NBANKS = cfg["GRP"], cfg["NBANKS"]
    NPC, NPCP, NG = meta["NPC"], meta["NPCP"], meta["NG"]
    nidx_tot, nb_tot = meta["nidx_tot"], meta["nb_tot"]
    units = meta["units"]
    XK = _ceil_div(IN, 128)
    assert OUT == 128 and GRP == 128
    last_w = NPC - (NG - 1) * GRP  # valid rows in the last group
    HALF = NPCP // 2               # rows per AG-half input

    nc = bacc.Bacc(
        "TRN2", target_bir_lowering=False, debug=False, num_devices=C
    )

    # ---- external inputs ----
    NXQ = 4  # x DMA split for earlier stage-B start
    xq = NPCP // NXQ
    assert NPCP % NXQ == 0
    xt = [
        nc.dram_tensor(f"xt{j}", [128, NPCP], BF16, kind="ExternalInput")
        for j in range(XK)
    ]
    wt = [
        nc.dram_tensor(f"wt{j}", [128, OUT], BF16, kind="ExternalInput")
        for j in range(XK)
    ]
    gidx_d = nc.dram_tensor("gidx", [128, nidx_tot // 16], I16, kind="ExternalInput")
    doff_d = nc.dram_tensor("doff", [128, nb_tot], F32, kind="ExternalInput")
    dego_d = nc.dram_tensor("dego", [128, NG], F32, kind="ExternalInput")
    degi_d = nc.dram_tensor("degi", [128, NG], F32, kind="ExternalInput")
    iota_d = nc.dram_tensor("iota", [128, GRP], BF16, kind="ExternalInput")
    gm_d = nc.dram_tensor("gm", [1, OUT], F32, kind="ExternalInput")
    bb_d = nc.dram_tensor("bb", [1, OUT], F32, kind="ExternalInput")
    onesc_d = nc.dram_tensor("onesc", [128, 1], BF16, kind="ExternalInput")
    onest_d = nc.dram_tensor("onest", [128, 1], BF16, kind="ExternalInput")
    onesr_d = nc.dram_tensor("onesr", [1, 128], F32, kind="ExternalInput")
    ident_d = nc.dram_tensor("ident", [128, 128], BF16, kind="ExternalInput")
    if b_nonzero:
        bt_d = nc.dram_tensor("bt", [1, OUT], F32, kind="ExternalInput")

    ypad_d = nc.dram_tensor("ypad", [NG * GRP, OUT], F32, kind="ExternalOutput")

    with tile.TileContext(nc) as tc:
        with (
            tc.tile_pool(name="const", bufs=1) as cpool,
            tc.tile_pool(name="dram", bufs=1, space="DRAM") as dpool,
            tc.tile_pool(name="agg", bufs=1) as apool,
            tc.tile_pool(name="mpool", bufs=8) as mpool,
            tc.tile_pool(name="etmp", bufs=4) as epool,
            tc.tile_pool(name="psg", bufs=4, space="PSUM") as pgpool,
            tc.tile_pool(name="psb", bufs=2, space="PSUM") as pbpool,
            tc.tile_pool(name="pstat", bufs=1, space="PSUM") as pspool,
        ):
            # ---- constants / small tiles ----
            iota_t = cpool.tile([128, GRP], BF16)
            dego_t = cpool.tile([128, NG], F32)
            degi_t = cpool.tile([128, NG], F32)
            nsrc_t = cpool.tile([128, NG], F32)
            ndst_t = cpool.tile([128, NG], F32)
            gm_t = cpool.tile([1, OUT], F32)
            bb_t = cpool.tile([1, OUT], F32)
            onesc_t = cpool.tile([128, 1], BF16)
            onest_t = cpool.tile([128, 1], BF16)
            onesr_t = cpool.tile([1, 128], F32)
            gidx_t = cpool.tile([128, nidx_tot // 16], I16)
            doff_t = cpool.tile([128, nb_tot], F32)
            ident_t = cpool.tile([128, 128], BF16)
            nc.sync.dma_start(ident_t[:], ident_d[:])

            nc.sync.dma_start(iota_t[:], iota_d[:])
            nc.sync.dma_start(dego_t[:], dego_d[:])
            nc.sync.dma_start(degi_t[:], degi_d[:])
            nc.sync.dma_start(gm_t[:], gm_d[:])
            nc.sync.dma_start(bb_t[:], bb_d[:])
            nc.sync.dma_start(onesc_t[:], onesc_d[:])
            nc.sync.dma_start(onest_t[:], onest_d[:])
            nc.sync.dma_start(onesr_t[:], onesr_d[:])
            nc.sync.dma_start(gidx_t[:], gidx_d[:])
            nc.sync.dma_start(doff_t[:], doff_d[:])
            if b_nonzero:
                bt_t = cpool.tile([1, OUT], F32)
                nc.sync.dma_start(bt_t[:], bt_d[:])

            # norms: rsqrt(max(deg, 1))
            for deg_t, norm_t in ((dego_t, nsrc_t), (degi_t, ndst_t)):
                nc.vector.tensor_scalar(
                    norm_t[:], deg_t[:], 1.0, None, op0=mybir.AluOpType.max
                )
                nc.vector.reciprocal(norm_t[:], norm_t[:])
                nc.scalar.activation(
                    norm_t[:], norm_t[:], mybir.ActivationFunctionType.Sqrt
                )

            # internal DRAM for collectives (even/odd node halves)
            _aspace = "Local" if cfg.get("NOCC") else "Shared"
            h_my_e = dpool.tile([HALF, OUT], BF16, name="h_my_e")
            h_my_o = dpool.tile([HALF, OUT], BF16, name="h_my_o")
            h_all_e = dpool.tile(
                [C * HALF, OUT], BF16, addr_space=_aspace, name="h_all_e"
            )
            h_all_o = dpool.tile(
                [C * HALF, OUT], BF16, addr_space=_aspace, name="h_all_o"
            )
            stats_in = dpool.tile([1, 2 * OUT], F32)
            stats_out = dpool.tile([1, 2 * OUT], F32, addr_space=_aspace)

            # relu(norm*agg) output, bf16, [128, NG, OUT]
            agg_t = apool.tile([128, NG, OUT], BF16)

            # ---- stage B: h = (x @ W) * norm_src, cast bf16, store to HBM
            # (staged in SBUF; 2 large DMAs instead of 98 small ones)
            with tc.tile_pool(name="xw", bufs=1) as xwp:
                xts = []
                wts = []
                for j in range(XK):
                    xtile = xwp.tile([128, NPCP], BF16, name=f"xt_s{j}")
                    wtile = xwp.tile([128, OUT], BF16, name=f"wt_s{j}")
                    for q in range(NXQ):
                        nc.sync.dma_start(
                            xtile[:, q * xq : (q + 1) * xq],
                            xt[j][:, q * xq : (q + 1) * xq],
                        )
                    nc.sync.dma_start(wtile[:], wt[j][:])
                    xts.append(xtile)
                    wts.append(wtile)
                hstage = xwp.tile([128, NG, OUT], BF16, name="hstage")
                for m in range(NG):
                    ps = pbpool.tile([128, OUT], F32, tag="hps")
                    for j in range(XK):
                        nc.tensor.matmul(
                            ps[:, :],
                            xts[j][:, m * GRP : (m + 1) * GRP],
                            wts[j][:, :],
                            start=(j == 0),
                            stop=(j == XK - 1),
                        )
                    nc.scalar.activation(
                        hstage[:, m, :],
                        ps[:, :],
                        mybir.ActivationFunctionType.Copy,
                        scale=nsrc_t[:, m : m + 1],
                    )
                # partitions 0:64 = even nodes of each group (loc = g*128+2q),
                # 64:128 = odd (x columns are host-permuted to match) ->
                # h_my_e row g*64+q = node loc 2r exactly
                hq = NG // 4
                for q in range(4):
                    a = q * hq
                    z = (q + 1) * hq if q < 3 else NG
                    nc.sync.dma_start(
                        h_my_e[a * 64 : z * 64, :].rearrange(
                            "(g p) f -> p g f", p=64
                        ),
                        hstage[0:64, a:z, :],
                    )
                    nc.sync.dma_start(
                        h_my_o[a * 64 : z * 64, :].rearrange(
                            "(g p) f -> p g f", p=64
                        ),
                        hstage[64:128, a:z, :],
                    )

            # ---- stage C: two AllGathers (even half, then odd half) ----
            for h_my_h, h_all_h in ((h_my_e, h_all_e), (h_my_o, h_all_o)):
                if cfg.get("NOCC"):
                    rep = (
                        h_my_h[:]
                        .rearrange("(o r) f -> o r f", o=1)
                        .to_broadcast((C, HALF, OUT))
                    )
                    nc.sync.dma_start(
                        h_all_h[:].rearrange("(o r) f -> o r f", o=C), rep
                    )
                else:
                    nc.gpsimd.collective_compute(
                        "AllGather",
                        mybir.AluOpType.bypass,
                        replica_groups=[list(range(C))],
                        ins=[h_my_h[:]],
                        outs=[h_all_h[:]],
                    )

            # interleaved bank views: bank b -> half b%2, row 2j + b//2
            h_banks = [
                (h_all_e if b % 2 == 0 else h_all_o)[:]
                .rearrange("(j k) f -> j (k f)", k=2)[
                    :, (b // 2) * OUT : (b // 2 + 1) * OUT
                ]
                for b in range(NBANKS)
            ]

            # ---- stage D: gather + one-hot matmul segmented sum ----
            # ---- stage E (inline): relu(psum*ndst) + BN partial sums ----
            # Gathers are batched per (chunk, bank); groups are processed
            # sequentially (their 4 bank runs back-to-back) so each PSUM bank
            # holds at most one pending accumulation group.
            ps_stat = pspool.tile([1, 2 * OUT], F32, name="ps_stat")
            ps_sum = ps_stat[:, 0:OUT]
            ps_sq = ps_stat[:, OUT : 2 * OUT]
            ndone = [0]  # groups completed (for BN-sum start/stop flags)

            def finish_group(g, ps_g):
                """relu + BN-sum accumulation for a completed group psum."""
                if b_nonzero:
                    tmp = epool.tile([128, OUT], F32, tag="etmp")
                    nc.vector.scalar_tensor_tensor(
                        tmp[:],
                        ps_g[:],
                        ndst_t[:, g : g + 1],
                        btile_t[:],
                        op0=mybir.AluOpType.mult,
                        op1=mybir.AluOpType.add,
                    )
                    nc.scalar.activation(
                        agg_t[:, g, :], tmp[:], mybir.ActivationFunctionType.Relu
                    )
                else:
                    nc.scalar.activation(
                        agg_t[:, g, :],
                        ps_g[:],
                        mybir.ActivationFunctionType.Relu,
                        scale=ndst_t[:, g : g + 1],
                    )
                ones = onesc_t if g < NG - 1 else onest_t
                i0 = ndone[0]
                # ps_sum/ps_sq share one bank = ONE accumulation group:
                # start only on the very first matmul, stop on the very last.
                nc.tensor.matmul(
                    ps_sum,
                    ones[:],
                    agg_t[:, g, :],
                    start=(i0 == 0),
                    stop=False,
                )
                sq = epool.tile([128, OUT], BF16, tag="esq")
                nc.scalar.activation(
                    sq[:], agg_t[:, g, :], mybir.ActivationFunctionType.Square
                )
                nc.tensor.matmul(
                    ps_sq,
                    ones[:],
                    sq[:],
                    start=False,
                    stop=(i0 == NG - 1),
                )
                ndone[0] += 1

            if b_nonzero:
                # replicate b across partitions once (PE broadcast)
                ps_b = pbpool.tile([128, OUT], F32, tag="hps", name="ps_b")
                btile_t = cpool.tile([128, OUT], F32)
                nc.tensor.matmul(ps_b[:], onesr_t[:], bt_t[:], start=True, stop=True)
                nc.scalar.activation(
                    btile_t[:], ps_b[:], mybir.ActivationFunctionType.Copy
                )

            run_blk = meta["run_blk"]
            chunks = meta["chunks"]
            nbmax = max(nb for _, _, nb in units)
            dstack = tc.tile_pool(name="gath", bufs=8)
            gpool = dstack.__enter__()
            phases = [(0, 2), (1, 3)]
            ui = 0
            for pi, ph in enumerate(phases):
                for ci, ch in enumerate(chunks):
                    gts = {}
                    for b in ph:
                        bank, t0, nblk = units[ui]
                        ui += 1
                        assert bank == b
                        Gt = gpool.tile(
                            [128, nbmax, OUT], BF16, tag="G", name=f"G{pi}_{ci}_{b}"
                        )
                        nc.gpsimd.dma_gather(
                            Gt[:, :nblk, :],
                            h_banks[b],
                            gidx_t[:, t0 * 8 : (t0 + nblk) * 8],
                            nblk * 128,
                            nblk * 128,
                            OUT,
                            elem_step=2 * OUT,
                            single_packet=False,
                        )
                        gts[b] = (Gt, t0)
                    for g in ch:
                        ps_g = pgpool.tile(
                            [128, OUT], F32, tag="aggps", name=f"ps{pi}_{g}"
                        )
                        if pi == 1:
                            # re-inject phase-A partial (spilled bf16)
                            nc.tensor.matmul(
                                ps_g[:],
                                ident_t[:],
                                agg_t[:, g, :],
                                start=True,
                                stop=False,
                            )
                        for bi, b in enumerate(ph):
                            Gt, t0 = gts[b]
                            blk0, nblk = run_blk[(b, g)]
                            for j in range(nblk):
                                t = blk0 + j
                                Mt = mpool.tile([128, GRP], BF16, tag="M")
                                nc.vector.tensor_scalar(
                                    Mt[:],
                                    iota_t[:],
                                    doff_t[:, t : t + 1],
                                    None,
                                    op0=mybir.AluOpType.is_equal,
                                )
                                nc.tensor.matmul(
                                    ps_g[:],
                                    Mt[:],
                                    Gt[:, t - t0, :],
                                    start=(pi == 0 and bi == 0 and j == 0),
                                    stop=(bi == 1 and j == nblk - 1),
                                )
                        if pi == 0:
                            # spill partial sum to agg_t (bf16), no relu yet
                            nc.scalar.activation(
                                agg_t[:, g, :],
                                ps_g[:],
                                mybir.ActivationFunctionType.Copy,
                            )
                        else:
                            finish_group(g, ps_g)
            dstack.__exit__(None, None, None)
            assert ndone[0] == NG

            # ---- stage F: AllReduce BN stats; build affine S/T tiles ----
            st_sb = cpool.tile([1, 2 * OUT], F32)
            nc.scalar.activation(
                st_sb[:, 0:OUT], ps_sum, mybir.ActivationFunctionType.Copy
            )
            nc.scalar.activation(
                st_sb[:, OUT : 2 * OUT], ps_sq, mybir.ActivationFunctionType.Copy
            )
            nc.sync.dma_start(stats_in[:], st_sb[:])
            if cfg.get("NOCC"):
                nc.sync.dma_start(stats_out[:], stats_in[:])
            else:
                nc.gpsimd.collective_compute(
                    "AllReduce",
                    mybir.AluOpType.add,
                    replica_groups=[list(range(C))],
                    ins=[stats_in[:]],
                    outs=[stats_out[:]],
                )
            st_rb = cpool.tile([1, 2 * OUT], F32)
            nc.sync.dma_start(st_rb[:], stats_out[:])

            mu = cpool.tile([1, OUT], F32)
            ex2 = cpool.tile([1, OUT], F32)
            var = cpool.tile([1, OUT], F32)
            srow = cpool.tile([1, OUT], F32)
            trow = cpool.tile([1, OUT], F32)
            inv_n = 1.0 / float(N)
            nc.scalar.activation(
                mu[:], st_rb[:, 0:OUT], mybir.ActivationFunctionType.Copy, scale=inv_n
            )
            nc.scalar.activation(
                ex2[:],
                st_rb[:, OUT : 2 * OUT],
                mybir.ActivationFunctionType.Copy,
                scale=inv_n,
            )
            nc.scalar.activation(var[:], mu[:], mybir.ActivationFunctionType.Square)
            nc.vector.tensor_sub(var[:], ex2[:], var[:])
            # var <- rsqrt(var + eps) (ACT Rsqrt is banned for accuracy)
            nc.scalar.activation(
                var[:],
                var[:],
                mybir.ActivationFunctionType.Copy,
                bias=float(cfg["EPS"]),
            )
            nc.vector.reciprocal(var[:], var[:])
            nc.scalar.activation(var[:], var[:], mybir.ActivationFunctionType.Sqrt)
            nc.vector.tensor_mul(srow[:], gm_t[:], var[:])
            nc.vector.tensor_mul(trow[:], mu[:], srow[:])
            nc.vector.tensor_sub(trow[:], bb_t[:], trow[:])

            S_t = cpool.tile([128, OUT], BF16)
            T_t = cpool.tile([128, OUT], BF16)
            ps_S = pgpool.tile([128, OUT], F32, tag="aggps", name="ps_S")
            ps_T = pgpool.tile([128, OUT], F32, tag="aggps", name="ps_T")
            nc.tensor.matmul(ps_S[:], onesr_t[:], srow[:], start=True, stop=True)
            nc.tensor.matmul(ps_T[:], onesr_t[:], trow[:], start=True, stop=True)
            nc.scalar.activation(S_t[:], ps_S[:], mybir.ActivationFunctionType.Copy)
            nc.scalar.activation(T_t[:], ps_T[:], mybir.ActivationFunctionType.Copy)

            # ---- stage G: y = hrelu * S + T (bf16), cast f32 on DMA out ----
            with tc.tile_pool(name="gtmp", bufs=2) as gpool2:
                GB = 14  # groups per batched op
                S_bc = (
                    S_t[:]
                    .rearrange("p (o f) -> p o f", o=1)
                    .to_broadcast((128, GB, OUT))
                )
                T_bc = (
                    T_t[:]
                    .rearrange("p (o f) -> p o f", o=1)
                    .to_broadcast((128, GB, OUT))
                )
                ypad_view = ypad_d[:].rearrange("(g p) f -> p g f", p=128)
                for g0 in range(0, NG, GB):
                    gw = min(GB, NG - g0)
                    tmp = gpool2.tile([128, GB, OUT], BF16, tag="gtmp")
                    nc.vector.tensor_mul(
                        tmp[:, :gw, :],
                        agg_t[:, g0 : g0 + gw, :],
                        S_bc if gw == GB else S_t[:]
                        .rearrange("p (o f) -> p o f", o=1)
                        .to_broadcast((128, gw, OUT)),
                    )
                    nc.vector.tensor_add(
                        agg_t[:, g0 : g0 + gw, :],
                        tmp[:, :gw, :],
                        T_bc if gw == GB else T_t[:]
                        .rearrange("p (o f) -> p o f", o=1)
                        .to_broadcast((128, gw, OUT)),
                    )
                    nc.gpsimd.dma_start(
                        ypad_view[:, g0 : g0 + gw, :],
                        agg_t[:, g0 : g0 + gw, :],
                    )

    nc.compile()
    return nc


def kernel(x, src, dst, W, b, gamma, beta):
    global LAST_RESULTS
    cfg = CFG
    N, E, IN, OUT, C = cfg["N"], cfg["E"], cfg["IN"], cfg["OUT"], cfg["NCORES"]
    GRP = cfg["GRP"]
    assert x.shape == (N, IN) and W.shape == (IN, OUT)
    assert src.shape == (E,) and dst.shape == (E,)

    b = np.asarray(b, np.float32)
    b_nonzero = bool(np.any(b != 0.0))
    meta, gidx_cores, dstoff_cores = _preprocess(cfg, src, dst)
    NPC, NPCP, NG = meta["NPC"], meta["NPCP"], meta["NG"]
    XK = _ceil_div(IN, 128)
    last_w = NPC - (NG - 1) * GRP
    # node permutation: within each 128-node group, evens first then odds
    perm = np.concatenate([np.arange(0, 128, 2), np.arange(1, 128, 2)])
    g_ = np.arange(NPCP) // 128
    p_ = np.arange(NPCP) % 128
    permn = g_ * 128 + perm[p_]          # source node (local) per padded col
    valid = permn < NPC

    nc = _build_nc(cfg, meta, b_nonzero=b_nonzero)

    xT = np.ascontiguousarray(np.asarray(x, np.float32).T)  # [IN, N]
    Wn = np.asarray(W, np.float32)
    import ml_dtypes

    iota = np.tile(np.arange(GRP, dtype=np.float32)[None, :], (128, 1)).astype(
        ml_dtypes.bfloat16
    )
    onesc = np.ones((128, 1), np.float32)
    onest = np.zeros((128, 1), np.float32)
    onest[:last_w] = 1.0
    onesr = np.ones((1, 128), np.float32)
    gm = np.asarray(gamma, np.float32)[None, :]
    bb = np.asarray(beta, np.float32)[None, :]

    in_maps = []
    for k in range(C):
        im = {
            "gidx": gidx_cores[k],
            "doff": dstoff_cores[k],
            "dego": _tile_major(
                np.where(
                    valid,
                    meta["deg_out"][k * NPC + np.minimum(permn, NPC - 1)],
                    np.float32(1.0),
                ).astype(np.float32),
                NG,
                GRP,
                np.float32(1.0),
            ),
            "degi": _tile_major(
                meta["deg_in"][k * NPC : (k + 1) * NPC], NG, GRP, np.float32(1.0)
            ),
            "iota": iota,
            "gm": gm,
            "bb": bb,
            "onesc": onesc.astype(ml_dtypes.bfloat16),
            "onest": onest.astype(ml_dtypes.bfloat16),
            "onesr": onesr,
            "ident": np.eye(128, dtype=np.float32).astype(ml_dtypes.bfloat16),
        }
        if b_nonzero:
            im["bt"] = b[None, :]
        for j in range(XK):
            xcols = np.zeros((128, NPCP), np.float32)
            xcols[:, valid] = xT[
                j * 128 : (j + 1) * 128, k * NPC + permn[valid]
            ]
            im[f"xt{j}"] = xcols.astype(ml_dtypes.bfloat16)
            im[f"wt{j}"] = np.ascontiguousarray(
                Wn[j * 128 : (j + 1) * 128, :]
            ).astype(ml_dtypes.bfloat16)
        in_maps.append(im)

    if cfg.get("SIM"):
        from concourse.bass_interp import MultiCoreSim

        sim = MultiCoreSim(nc, num_cores=C)
        for k, core_sim in sim.cores.items():
            for name, val in in_maps[k].items():
                core_sim.tensor(name)[:] = val
        sim.simulate()
        y = np.empty((N, OUT), np.float32)
        for k in range(C):
            y[k * NPC : (k + 1) * NPC] = sim.cores[k].tensor("ypad")[:NPC]
        return y

    global LAST_NC, LAST_RUN_S
    LAST_NC = nc
    import time as _time

    _t0 = _time.time()
    res = bass_utils.run_bass_kernel_spmd(
        nc,
        in_maps,
        core_ids=list(range(C)),
        trace=cfg.get("TRACE", False),
    )
    LAST_RUN_S = _time.time() - _t0
    LAST_RESULTS = res

    y = np.empty((N, OUT), np.float32)
    for k in range(C):
        y[k * NPC : (k + 1) * NPC] = res.results[k]["ypad"][:NPC]
    return y
